# revision 24
# baseline (speedup 1.0000x reference)
"""DiagLinear (block-diagonal linear + output interleave + bias) on 8 TRN2 cores.

Reference computation (fp32):
    x:   (B=8, S=2048, P*DIN=4096)
    w:   (P=16, DOUT=256, DIN=256)
    b:   (4096,)
    y[b, s, o*P + p] = sum_i x[b, s, p*DIN + i] * w[p, o, i]  + bias[o*P+p]

Sharding: data parallel over the batch dim - core c computes batch c.

Numerics: x and w are rounded to bf16 on the host and the matmul runs a
single bf16 pass (fp32 PSUM accumulation); y is stored bf16 and upcast on
the host.  Measured end-to-end rel err ~3e-3 against the fp32 reference
(gate is 2e-2).

Feature-major schedule (v2).  The kernel computes yT[f, t] with output
features on partitions and tokens on the free dim:

  - w[p] chunk [i128, o128] is the stationary operand; x chunk [i128, T]
    streams through.  psum[o, t] accumulates the 2 K-chunks.
  - bias is a per-partition scalar column, folded into the single
    PSUM->SBUF drain op (ACT activation-with-bias / DVE tensor_scalar_add,
    alternating per stripe).  No separate bias pass, no interleave copy:
    the (o, p) output interleave is pure host-side reshaping of the
    block-grouped output rows r = p*256 + o.
  - All DMA is fully contiguous 512 KiB transfers (4 KiB per partition
    line): 32 x-chunk loads, 32 y-stripe stores.  x loads and y stores
    share the sync HWDGE ring in FIFO order, so x is strictly prioritized
    and the SDMA engines never sit idle; w + bias ride the scalar ring.

Per-core totals: DMA 34.3 MiB (~95 us at 368 GB/s), PE ~68 us, drain
engines ~35 us each -> DMA-roofline-bound schedule.
"""

import contextlib
import ctypes
import sys
import types

import numpy as np

from concourse import bass, mybir, tile
from concourse.bass_utils import run_bass_kernel_spmd


def _install_ntff_shim():
    """Provide antenv.axon_hooks (missing in this image) so trace=True can
    capture NTFF profiles via the axon .so.  Only used when profiling."""
    if "antenv.axon_hooks" in sys.modules:
        return
    so = "/opt/axon/libaxon_pjrt.so"
    try:
        lib = ctypes.CDLL(so)
        lib.axon_start_nrt_profile.argtypes = [
            ctypes.POINTER(ctypes.c_int64),
            ctypes.c_size_t,
        ]
        lib.axon_start_nrt_profile.restype = ctypes.c_int64
        lib.axon_stop_nrt_profile.argtypes = [ctypes.c_char_p]
        lib.axon_stop_nrt_profile.restype = ctypes.c_int64
    except (OSError, AttributeError):
        return

    @contextlib.contextmanager
    def hook(output_dir, device_ids):
        import jax

        jax.devices()
        if device_ids:
            ids = (ctypes.c_int64 * len(device_ids))(*device_ids)
            rc = lib.axon_start_nrt_profile(ids, len(device_ids))
        else:
            rc = lib.axon_start_nrt_profile(None, 0)
        if rc != 0:
            raise RuntimeError(f"axon_start_nrt_profile rc={rc}")
        try:
            yield
        finally:
            n = lib.axon_stop_nrt_profile(str(output_dir).encode())
            print(f"ntff profile: {n} file(s) -> {output_dir}", file=sys.stderr)

    mod = types.ModuleType("antenv.axon_hooks")
    mod.get_axon_ntff_profile_hook = lambda: hook
    mod.set_axon_ntff_profile_hook = lambda h: None
    sys.modules["antenv.axon_hooks"] = mod


P = 16
DIN = 256
DOUT = 256
B = 8
S = 2048
D = P * DIN  # 4096
N_CHUNKS = D // 128  # 32 feature chunks of 128
TB = 512  # token block (one PSUM bank of fp32; matmul N>512 crashes the device)
N_TB = S // TB  # 4
F32 = mybir.dt.float32
BF16 = mybir.dt.bfloat16


def _split_multi_waits(nc, max_waits=1):
    """This container's walrus build accepts at most one sync-wait per
    instruction; Tile attaches several.  Move the surplus onto dedicated
    single-wait EventSemaphore instructions right before the instruction
    on the same engine (same semantics: the engine is serial)."""
    n_split = 0
    for f in nc.m.functions:
        for bb in f.blocks:
            new_insts = []
            for inst in bb.instructions:
                si = inst.sync_info
                if si is not None and si.on_wait and len(si.on_wait) > max_waits:
                    waits = list(si.on_wait)
                    extra, keep = waits[:-max_waits], waits[-max_waits:]
                    for k, w in enumerate(extra):
                        nop = mybir.InstEventSemaphore(
                            name=f"{inst.name}-wsplit-{k}",
                            engine=inst.engine,
                            sync_info=mybir.SyncInfo(on_wait=[w], on_update=[]),
                        )
                        nc.register_instruction(nop)
                        new_insts.append(nop)
                        n_split += 1
                    inst.sync_info = mybir.SyncInfo(
                        on_wait=keep, on_update=list(si.on_update or [])
                    )
                new_insts.append(inst)
            bb.instructions[:] = new_insts
    return n_split


def build_nc():
    nc = bass.Bass()
    x_d = nc.declare_dram_parameter("xt", [D, S], BF16, isOutput=False)
    # stationary tiles: w_d[i, ((p*2+c)*2+h)*128 + m] = w[p, h*128+m, c*128+i]
    w_d = nc.declare_dram_parameter("w", [128, 64 * 128], BF16, isOutput=False)
    # bias columns: b_d[oo, p*2+h] = bias[(h*128+oo)*16 + p]
    b_d = nc.declare_dram_parameter("bias_ph", [128, 32], F32, isOutput=False)
    # block-grouped output rows: y_d[p*256 + o, t]
    y_d = nc.declare_dram_parameter("y", [D, S], BF16, isOutput=True)

    with tile.TileContext(nc) as tc:
        with (
            tc.tile_pool(name="const", bufs=1) as const_pool,
            tc.tile_pool(name="xc", bufs=P) as pool_x,
            tc.tile_pool(name="y_sb", bufs=14) as pool_y,
            tc.tile_pool(name="ps", bufs=8, space="PSUM") as pool_ps,
        ):
            # ALL x/w descriptor generation lives on the sync sequencer: it
            # runs no compute, so HWDGE ring backpressure can stall it
            # harmlessly.  The ACT sequencer must NOT generate x descriptors
            # -- a full ring blocks its FIFO, which blocks the PSUM drains,
            # which deadlocks the PE on PSUM WAR for ~15 us.
            w_tiles = []
            for k in range(4):
                wt_k = const_pool.tile([128, 16 * 128], BF16, tag=f"wt{k}")
                w_tiles.append(wt_k)
            nc.sync.dma_start(w_tiles[0][:], w_d[:, 0:2048])
            bias_sb = const_pool.tile([128, 32], F32)
            nc.gpsimd.dma_start(bias_sb[:], b_d[:])

            def w_ap(p, c, h):
                idx = (p * 2 + c) * 2 + h
                return w_tiles[idx // 16][:, (idx % 16) * 128 : (idx % 16 + 1) * 128]

            # x: one 1 MiB load per block p carrying both K-chunks (4 KiB
            # contiguous per partition line), split across BOTH HWDGE rings
            # (sync/scalar) so x rides ~2/3 of SDMA round-robin bandwidth
            # and descriptor gen is parallelized.  p0/p1 split per-chunk so
            # the first matmuls fire ~3 us sooner.
            xt = []
            for j in range(P):
                xt_j = pool_x.tile([128, 2, S], BF16)
                if j < 2:
                    for c in range(2):
                        nc.sync.dma_start(
                            xt_j[:, c, :],
                            x_d[j * 256 + c * 128 : j * 256 + (c + 1) * 128, :],
                        )
                else:
                    nc.sync.dma_start(
                        xt_j[:],
                        x_d[j * 256 : (j + 1) * 256, :].rearrange(
                            "(c q) t -> q c t", c=2
                        ),
                    )
                xt.append(xt_j)
                # w parts 1-3 slot in so w_k lands well before p=4k computes
                if j in (1, 4, 7):
                    k = {1: 1, 4: 2, 7: 3}[j]
                    nc.sync.dma_start(
                        w_tiles[k][:], w_d[:, k * 2048 : (k + 1) * 2048]
                    )

            for p in range(P):
                for h in range(2):
                    s_idx = p * 2 + h
                    y_sb = pool_y.tile([128, S], BF16)
                    ps = [
                        pool_ps.tile([128, TB], F32, name="ps") for _ in range(N_TB)
                    ]
                    # c-outer: consecutive matmuls share the stationary tile
                    for c in range(2):
                        for tb in range(N_TB):
                            nc.tensor.matmul(
                                ps[tb][:],
                                w_ap(p, c, h),
                                xt[p][:, c, tb * TB : (tb + 1) * TB],
                                start=(c == 0),
                                stop=(c == 1),
                            )
                    bcol = bias_sb[:, s_idx : s_idx + 1]
                    for tb in range(N_TB):
                        dst = y_sb[:, tb * TB : (tb + 1) * TB]
                        # alternate drains per token-block so neither engine's
                        # FIFO serializes a whole stripe's PSUM return
                        if (s_idx + tb) % 2 == 0:
                            nc.vector.tensor_scalar_add(dst, ps[tb][:], bcol)
                        else:
                            nc.scalar.activation(
                                dst,
                                ps[tb][:],
                                mybir.ActivationFunctionType.Identity,
                                bias=bcol,
                                scale=1.0,
                            )
                    # y ring assignment: early stripes (0-15) drain on the
                    # otherwise-idle gpsimd SWDGE ring concurrently with x;
                    # late stripes are held in SBUF (16-buf y pool) and flush
                    # on BOTH HWDGE rings once their x loads have drained, so
                    # the tail runs multiple queues (per-DMA HBM-write-receipt
                    # latency overlaps) at full rate
                    if s_idx < 16:
                        y_eng = nc.gpsimd
                    elif s_idx < 24:
                        y_eng = nc.sync
                    else:
                        y_eng = nc.scalar
                    y_eng.dma_start(
                        y_d[s_idx * 128 : (s_idx + 1) * 128, :], y_sb[:]
                    )

    _split_multi_waits(nc)
    return nc


def _host_weight(weight):
    # w_host[i, ((p*2+c)*2+h)*128 + m] = weight[p, h*128+m, c*128+i]
    wt = weight.reshape(P, 2, 128, 2, 128)  # [p, h, m, c, i]
    return np.ascontiguousarray(
        wt.transpose(4, 0, 3, 1, 2).reshape(128, 64 * 128)
    )


def _host_bias(bias):
    # b_host[oo, p*2+h] = bias[(h*128+oo)*16 + p]
    bb = bias.reshape(2, 128, P)  # [h, oo, p]
    return np.ascontiguousarray(bb.transpose(1, 2, 0).reshape(128, 2 * P))


def kernel(inputs, weight, bias, _trace=False):
    import ml_dtypes

    inputs = np.asarray(inputs, dtype=np.float32)
    weight = np.asarray(weight, dtype=np.float32)
    bias = np.asarray(bias, dtype=np.float32)
    assert inputs.shape == (B, S, D)

    if _trace:
        _install_ntff_shim()
    nc = build_nc()
    w_host = _host_weight(weight).astype(ml_dtypes.bfloat16)
    b_host = _host_bias(bias).astype(np.float32)
    common = {"bias_ph": b_host, "w": w_host}
    in_maps = [
        {"xt": np.ascontiguousarray(inputs[c].astype(ml_dtypes.bfloat16).T), **common}
        for c in range(B)
    ]
    res = run_bass_kernel_spmd(nc, in_maps, core_ids=list(range(8)), trace=_trace)
    out = np.empty((B, S, D), dtype=np.float32)
    for c in range(B):
        yc = np.asarray(res.results[c]["y"]).astype(np.float32)  # [D, S]
        # y_full[s, o*16+p] = yc[p*256+o, s]
        out[c] = yc.reshape(P, DOUT, S).transpose(2, 1, 0).reshape(S, D)
    if _trace:
        kernel.last_exec_time_ns = res.exec_time_ns
        kernel.last_results = res
    return out


# revision 26
# speedup vs baseline: 1.0102x; 1.0102x over previous
"""DiagLinear (block-diagonal linear + output interleave + bias) on 8 TRN2 cores.

Reference computation (fp32):
    x:   (B=8, S=2048, P*DIN=4096)
    w:   (P=16, DOUT=256, DIN=256)
    b:   (4096,)
    y[b, s, o*P + p] = sum_i x[b, s, p*DIN + i] * w[p, o, i]  + bias[o*P+p]

Sharding: data parallel over the batch dim - core c computes batch c.

Numerics: x and w are rounded to bf16 on the host and the matmul runs a
single bf16 pass (fp32 PSUM accumulation); y is stored bf16 and upcast on
the host.  Measured end-to-end rel err ~3e-3 against the fp32 reference
(gate is 2e-2).

Feature-major schedule (v2).  The kernel computes yT[f, t] with output
features on partitions and tokens on the free dim:

  - w[p] chunk [i128, o128] is the stationary operand; x chunk [i128, T]
    streams through.  psum[o, t] accumulates the 2 K-chunks.
  - bias is a per-partition scalar column, folded into the single
    PSUM->SBUF drain op (ACT activation-with-bias / DVE tensor_scalar_add,
    alternating per stripe).  No separate bias pass, no interleave copy:
    the (o, p) output interleave is pure host-side reshaping of the
    block-grouped output rows r = p*256 + o.
  - All DMA is fully contiguous 512 KiB transfers (4 KiB per partition
    line): 32 x-chunk loads, 32 y-stripe stores.  x loads and y stores
    share the sync HWDGE ring in FIFO order, so x is strictly prioritized
    and the SDMA engines never sit idle; w + bias ride the scalar ring.

Per-core totals: DMA 34.3 MiB (~95 us at 368 GB/s), PE ~68 us, drain
engines ~35 us each -> DMA-roofline-bound schedule.
"""

import contextlib
import ctypes
import sys
import types

import numpy as np

from concourse import bass, mybir, tile
from concourse.bass_utils import run_bass_kernel_spmd


def _install_ntff_shim():
    """Provide antenv.axon_hooks (missing in this image) so trace=True can
    capture NTFF profiles via the axon .so.  Only used when profiling."""
    if "antenv.axon_hooks" in sys.modules:
        return
    so = "/opt/axon/libaxon_pjrt.so"
    try:
        lib = ctypes.CDLL(so)
        lib.axon_start_nrt_profile.argtypes = [
            ctypes.POINTER(ctypes.c_int64),
            ctypes.c_size_t,
        ]
        lib.axon_start_nrt_profile.restype = ctypes.c_int64
        lib.axon_stop_nrt_profile.argtypes = [ctypes.c_char_p]
        lib.axon_stop_nrt_profile.restype = ctypes.c_int64
    except (OSError, AttributeError):
        return

    @contextlib.contextmanager
    def hook(output_dir, device_ids):
        import jax

        jax.devices()
        if device_ids:
            ids = (ctypes.c_int64 * len(device_ids))(*device_ids)
            rc = lib.axon_start_nrt_profile(ids, len(device_ids))
        else:
            rc = lib.axon_start_nrt_profile(None, 0)
        if rc != 0:
            raise RuntimeError(f"axon_start_nrt_profile rc={rc}")
        try:
            yield
        finally:
            n = lib.axon_stop_nrt_profile(str(output_dir).encode())
            print(f"ntff profile: {n} file(s) -> {output_dir}", file=sys.stderr)

    mod = types.ModuleType("antenv.axon_hooks")
    mod.get_axon_ntff_profile_hook = lambda: hook
    mod.set_axon_ntff_profile_hook = lambda h: None
    sys.modules["antenv.axon_hooks"] = mod


P = 16
DIN = 256
DOUT = 256
B = 8
S = 2048
D = P * DIN  # 4096
N_CHUNKS = D // 128  # 32 feature chunks of 128
TB = 512  # token block (one PSUM bank of fp32; matmul N>512 crashes the device)
N_TB = S // TB  # 4
F32 = mybir.dt.float32
BF16 = mybir.dt.bfloat16


def _split_multi_waits(nc, max_waits=1):
    """This container's walrus build accepts at most one sync-wait per
    instruction; Tile attaches several.  Move the surplus onto dedicated
    single-wait EventSemaphore instructions right before the instruction
    on the same engine (same semantics: the engine is serial)."""
    n_split = 0
    for f in nc.m.functions:
        for bb in f.blocks:
            new_insts = []
            for inst in bb.instructions:
                si = inst.sync_info
                if si is not None and si.on_wait and len(si.on_wait) > max_waits:
                    waits = list(si.on_wait)
                    extra, keep = waits[:-max_waits], waits[-max_waits:]
                    for k, w in enumerate(extra):
                        nop = mybir.InstEventSemaphore(
                            name=f"{inst.name}-wsplit-{k}",
                            engine=inst.engine,
                            sync_info=mybir.SyncInfo(on_wait=[w], on_update=[]),
                        )
                        nc.register_instruction(nop)
                        new_insts.append(nop)
                        n_split += 1
                    inst.sync_info = mybir.SyncInfo(
                        on_wait=keep, on_update=list(si.on_update or [])
                    )
                new_insts.append(inst)
            bb.instructions[:] = new_insts
    return n_split


def build_nc():
    nc = bass.Bass()
    x_d = nc.declare_dram_parameter("xt", [D, S], BF16, isOutput=False)
    # stationary tiles: w_d[i, ((p*2+c)*2+h)*128 + m] = w[p, h*128+m, c*128+i]
    w_d = nc.declare_dram_parameter("w", [128, 64 * 128], BF16, isOutput=False)
    # bias columns: b_d[oo, p*2+h] = bias[(h*128+oo)*16 + p]
    b_d = nc.declare_dram_parameter("bias_ph", [128, 32], F32, isOutput=False)
    # block-grouped output rows: y_d[p*256 + o, t]
    y_d = nc.declare_dram_parameter("y", [D, S], BF16, isOutput=True)

    with tile.TileContext(nc) as tc:
        with (
            tc.tile_pool(name="const", bufs=1) as const_pool,
            tc.tile_pool(name="xc", bufs=P) as pool_x,
            tc.tile_pool(name="y_sb", bufs=14) as pool_y,
            tc.tile_pool(name="ps", bufs=8, space="PSUM") as pool_ps,
        ):
            # ALL x/w descriptor generation lives on the sync sequencer: it
            # runs no compute, so HWDGE ring backpressure can stall it
            # harmlessly.  The ACT sequencer must NOT generate x descriptors
            # -- a full ring blocks its FIFO, which blocks the PSUM drains,
            # which deadlocks the PE on PSUM WAR for ~15 us.
            w_tiles = []
            for k in range(4):
                wt_k = const_pool.tile([128, 16 * 128], BF16, tag=f"wt{k}")
                w_tiles.append(wt_k)
            nc.sync.dma_start(w_tiles[0][:], w_d[:, 0:2048])
            bias_sb = const_pool.tile([128, 32], F32)
            nc.gpsimd.dma_start(bias_sb[:], b_d[:])

            def w_ap(p, c, h):
                idx = (p * 2 + c) * 2 + h
                return w_tiles[idx // 16][:, (idx % 16) * 128 : (idx % 16 + 1) * 128]

            # x: one 1 MiB load per block p carrying both K-chunks (4 KiB
            # contiguous per partition line), split across BOTH HWDGE rings
            # (sync/scalar) so x rides ~2/3 of SDMA round-robin bandwidth
            # and descriptor gen is parallelized.  p0/p1 split per-chunk so
            # the first matmuls fire ~3 us sooner.
            # x rides the two compute-free sequencers: evens on sync (HWDGE),
            # odds on gpsimd (SWDGE) -- so x gets ~2 of 3 ring shares and no
            # compute engine's FIFO can be blocked by x descriptor gen
            xt = []
            for j in range(P):
                xt_j = pool_x.tile([128, 2, S], BF16)
                eng = nc.sync if j % 2 == 0 else nc.gpsimd
                if j < 2:
                    for c in range(2):
                        eng.dma_start(
                            xt_j[:, c, :],
                            x_d[j * 256 + c * 128 : j * 256 + (c + 1) * 128, :],
                        )
                else:
                    eng.dma_start(
                        xt_j[:],
                        x_d[j * 256 : (j + 1) * 256, :].rearrange(
                            "(c q) t -> q c t", c=2
                        ),
                    )
                xt.append(xt_j)
                # w parts 1-3 slot in so w_k lands well before p=4k computes
                if j in (1, 4, 7):
                    k = {1: 1, 4: 2, 7: 3}[j]
                    nc.sync.dma_start(
                        w_tiles[k][:], w_d[:, k * 2048 : (k + 1) * 2048]
                    )

            for p in range(P):
                for h in range(2):
                    s_idx = p * 2 + h
                    y_sb = pool_y.tile([128, S], BF16)
                    ps = [
                        pool_ps.tile([128, TB], F32, name="ps") for _ in range(N_TB)
                    ]
                    # c-outer: consecutive matmuls share the stationary tile
                    for c in range(2):
                        for tb in range(N_TB):
                            nc.tensor.matmul(
                                ps[tb][:],
                                w_ap(p, c, h),
                                xt[p][:, c, tb * TB : (tb + 1) * TB],
                                start=(c == 0),
                                stop=(c == 1),
                            )
                    bcol = bias_sb[:, s_idx : s_idx + 1]
                    for tb in range(N_TB):
                        dst = y_sb[:, tb * TB : (tb + 1) * TB]
                        # alternate drains per token-block so neither engine's
                        # FIFO serializes a whole stripe's PSUM return
                        if (s_idx + tb) % 2 == 0:
                            nc.vector.tensor_scalar_add(dst, ps[tb][:], bcol)
                        else:
                            nc.scalar.activation(
                                dst,
                                ps[tb][:],
                                mybir.ActivationFunctionType.Identity,
                                bias=bcol,
                                scale=1.0,
                            )
                    # y ring assignment: early stripes (0-15) go on the ACT
                    # ring -- their descriptor gens each wait only the drains
                    # issued immediately before them in the same FIFO, so they
                    # fire promptly and drain concurrently with x.  Late
                    # stripes flush on gpsimd/sync behind those rings' x
                    # loads, so the tail runs multiple queues (per-DMA
                    # HBM-write-receipt latency overlaps) at full rate.
                    if s_idx < 16:
                        y_eng = nc.scalar
                    elif s_idx < 24:
                        y_eng = nc.gpsimd
                    else:
                        y_eng = nc.sync
                    y_eng.dma_start(
                        y_d[s_idx * 128 : (s_idx + 1) * 128, :], y_sb[:]
                    )

    _split_multi_waits(nc)
    return nc


def _host_weight(weight):
    # w_host[i, ((p*2+c)*2+h)*128 + m] = weight[p, h*128+m, c*128+i]
    wt = weight.reshape(P, 2, 128, 2, 128)  # [p, h, m, c, i]
    return np.ascontiguousarray(
        wt.transpose(4, 0, 3, 1, 2).reshape(128, 64 * 128)
    )


def _host_bias(bias):
    # b_host[oo, p*2+h] = bias[(h*128+oo)*16 + p]
    bb = bias.reshape(2, 128, P)  # [h, oo, p]
    return np.ascontiguousarray(bb.transpose(1, 2, 0).reshape(128, 2 * P))


def kernel(inputs, weight, bias, _trace=False):
    import ml_dtypes

    inputs = np.asarray(inputs, dtype=np.float32)
    weight = np.asarray(weight, dtype=np.float32)
    bias = np.asarray(bias, dtype=np.float32)
    assert inputs.shape == (B, S, D)

    if _trace:
        _install_ntff_shim()
    nc = build_nc()
    w_host = _host_weight(weight).astype(ml_dtypes.bfloat16)
    b_host = _host_bias(bias).astype(np.float32)
    common = {"bias_ph": b_host, "w": w_host}
    in_maps = [
        {"xt": np.ascontiguousarray(inputs[c].astype(ml_dtypes.bfloat16).T), **common}
        for c in range(B)
    ]
    res = run_bass_kernel_spmd(nc, in_maps, core_ids=list(range(8)), trace=_trace)
    out = np.empty((B, S, D), dtype=np.float32)
    for c in range(B):
        yc = np.asarray(res.results[c]["y"]).astype(np.float32)  # [D, S]
        # y_full[s, o*16+p] = yc[p*256+o, s]
        out[c] = yc.reshape(P, DOUT, S).transpose(2, 1, 0).reshape(S, D)
    if _trace:
        kernel.last_exec_time_ns = res.exec_time_ns
        kernel.last_results = res
    return out
